# revision 16
# baseline (speedup 1.0000x reference)
"""Trainium2 Bass kernel for nn_NeuralTensorDiagLayer.

Computes out = tanh(concat([e1, e2], -1) @ V + diag + b) where
diag[k] = (sum_b(e1*e2) @ W[k]) / (B*D), broadcast over batch.

Sharding (8 NeuronCores, 2D: 4 batch groups x 2 k_out halves):
  - Core c handles batch rows [1024*(c//2), 1024*(c//2+1)) and k_out
    columns [1024*(c%2), 1024*(c%2+1)). Compared to pure batch-parallel,
    this halves the dominant V stream (16 MiB instead of 32 MiB per core;
    38 MiB total HBM traffic per core vs 46 MiB).
  - e1/e2 shards are fed pre-transposed to [feat, batch] by the host and
    held resident in SBUF (reused by both the matmul and the diag path);
    V arrives feature-major so no device transposes are needed.
  - diag: fused-on-DVE partial sum_b(e1*e2) per core, AllReduce over all
    8 cores (each batch row is counted twice -> 0.5 folded into the scale),
    then each core computes a 256-row diag slice against its W^T shard and
    an AllGather over the subgroups [[0,2,4,6],[1,3,5,7]] assembles each
    k_out half. The diag slice assignment is permuted (sc = (c%2)*4 + c//2,
    applied host-side when slicing W and b) so each subgroup gathers
    exactly its half in rank order - the device program stays SPMD-identical.
  - Main matmul runs in float32r (TensorE reduced-precision fp32 mode, 4x
    the fp32 throughput at ~12-bit mantissa accuracy); PSUM accumulation is
    fp32. V tiles are reused across both 512-wide batch chunks so each V
    element is read exactly once. PSUM is drained by DVE copies so the
    TensorEngine never waits on the diag collective chain; tanh+bias runs
    on the ScalarEngine afterwards.

Output is produced transposed ([k_out, batch] per core); the host
transposes/concats the 4x2 block grid back to (B, K).
"""

import os
import sys

for _p in ("/opt/trn_rl_repo", "/root/.axon_site/_ro/trn_rl_repo"):
    if os.path.isdir(_p) and _p not in sys.path:
        sys.path.append(_p)

import numpy as np

N_CORES = 8
B, D, K_OUT = 4096, 2048, 2048
FEAT = 2 * D
BG, KH = 4, 2                 # batch groups x kout halves
BPC = B // BG                 # 1024 batch rows per core
KHC = K_OUT // KH             # 1024 kout cols per core
KPC = K_OUT // N_CORES        # 256 diag rows per core
FT = FEAT // 128              # 32 feature tiles
DT = D // 128                 # 16 e1-space feature tiles
KTL = KHC // 128              # 8 local kout tiles
KGROUPS = (3, 3, 2)           # kout tile groups (2*g live PSUM banks)
DIAG_SCALE = 0.5 / (B * D)    # 0.5: the 8-core allreduce double-counts rows

_CACHE = {}


def _build_nc():
    import concourse.bacc as bacc
    import concourse.tile as tile
    import concourse.mybir as mybir

    repeat = int(os.environ.get("KERNEL_REPEAT", "1"))
    no_cc = bool(int(os.environ.get("KERNEL_NO_CC", "0")))
    dt = mybir.dt
    nc = bacc.Bacc("TRN2", target_bir_lowering=False, debug=False,
                   num_devices=N_CORES)

    e1t = nc.dram_tensor("e1t", [D, BPC], dt.float32r, kind="ExternalInput").ap()
    e2t = nc.dram_tensor("e2t", [D, BPC], dt.float32r, kind="ExternalInput").ap()
    v = nc.dram_tensor("v", [FEAT, KHC], dt.float32r, kind="ExternalInput").ap()
    wt = nc.dram_tensor("wt", [D, KPC], dt.float32, kind="ExternalInput").ap()
    bvec = nc.dram_tensor("bvec", [1, KPC], dt.float32, kind="ExternalInput").ap()
    out = nc.dram_tensor("out", [KHC, BPC], dt.float32, kind="ExternalOutput").ap()

    core_ids = list(range(N_CORES))
    ag_groups = [[0, 2, 4, 6], [1, 3, 5, 7]]

    with tile.TileContext(nc) as tc:
        with tc.tile_pool(name="xpool", bufs=1) as xpool, \
             tc.tile_pool(name="vpool", bufs=4) as vpool, \
             tc.tile_pool(name="wpool", bufs=4) as wpool, \
             tc.tile_pool(name="spool", bufs=1) as spool, \
             tc.tile_pool(name="scratch", bufs=2) as scratch, \
             tc.tile_pool(name="stage", bufs=1) as stage_pool, \
             tc.tile_pool(name="opool", bufs=2) as opool, \
             tc.tile_pool(name="psum", bufs=7, space="PSUM") as pp, \
             tc.tile_pool(name="psd", bufs=1, space="PSUM") as ppd, \
             tc.tile_pool(name="dram", bufs=1, space="DRAM") as dram:

            # ---- resident X^T = [e1^T ; e2^T] : 32 tiles of [128, BPC] ----
            x_all = xpool.tile([128, FT * BPC], dt.float32r)
            for j in range(DT):
                nc.sync.dma_start(x_all[:, j * BPC:(j + 1) * BPC],
                                  e1t[j * 128:(j + 1) * 128, :])
            for j in range(DT):
                jj = DT + j
                nc.sync.dma_start(x_all[:, jj * BPC:(jj + 1) * BPC],
                                  e2t[j * 128:(j + 1) * 128, :])

            # ---- partial s = sum_batch(e1*e2) on DVE ----
            # (tensor_tensor_reduce would fuse these but crashes the device)
            s_sb = spool.tile([128, DT], dt.float32)
            for j in range(DT):
                prod = scratch.tile([128, BPC], dt.float32, tag="prod",
                                    name=f"prod{j}")
                nc.vector.tensor_mul(
                    prod[:],
                    x_all[:, j * BPC:(j + 1) * BPC].bitcast(dt.float32),
                    x_all[:, (DT + j) * BPC:(DT + j + 1) * BPC].bitcast(dt.float32))
                nc.vector.tensor_reduce(s_sb[:, j:j + 1], prod[:],
                                        mybir.AxisListType.X,
                                        mybir.AluOpType.add)

            # ---- AllReduce s over all cores (8 KiB) ----
            s_in = dram.tile([128, DT], dt.float32)
            s_out = dram.tile([128, DT], dt.float32,
                              addr_space="Local" if no_cc else "Shared")
            nc.sync.dma_start(s_in[:], s_sb[:])
            if no_cc:
                nc.sync.dma_start(s_out[:], s_in[:])
            else:
                nc.gpsimd.collective_compute(
                    "AllReduce", mybir.AluOpType.add,
                    replica_groups=[core_ids],
                    ins=[s_in.opt()], outs=[s_out.opt()])
            s_r = spool.tile([128, DT], dt.float32, name="s_r")
            nc.sync.dma_start(s_r[:], s_out[:])

            # ---- diag slice: [1, KPC] = s @ wt (fp32 matmuls, M=1) ----
            b_sb = spool.tile([1, KPC], dt.float32, name="b_sb")
            nc.sync.dma_start(b_sb[:], bvec[:])
            diag_sb = spool.tile([1, KPC], dt.float32, name="diag_sb")
            ps_d = ppd.tile([1, KPC], dt.float32)
            for j in range(DT):
                wt_t = wpool.tile([128, KPC], dt.float32, tag="wt", name=f"wt{j}")
                nc.sync.dma_start(wt_t[:], wt[j * 128:(j + 1) * 128, :])
                nc.tensor.matmul(ps_d[:], s_r[:, j:j + 1], wt_t[:],
                                 start=(j == 0), stop=(j == DT - 1))
            nc.vector.tensor_scalar_mul(diag_sb[:], ps_d[:], DIAG_SCALE)
            nc.vector.tensor_add(diag_sb[:], diag_sb[:], b_sb[:])

            # ---- AllGather diag within the kout-half subgroup (1 KiB) ----
            d_in = dram.tile([1, KPC], dt.float32, name="d_in")
            d_out = dram.tile([KTL, 128], dt.float32, name="d_out")
            nc.sync.dma_start(d_in[:], diag_sb[:])
            if no_cc:
                for i in range(4):
                    nc.sync.dma_start(
                        d_out[2 * i:2 * i + 2, :],
                        d_in[:].rearrange("a (x p) -> (a x) p", p=128))
            else:
                nc.gpsimd.collective_compute(
                    "AllGather", mybir.AluOpType.bypass,
                    replica_groups=ag_groups,
                    ins=[d_in.opt()], outs=[d_out.opt()])
            # load as [128, KTL]: partition p, col k  <-  diag_half[k*128 + p]
            diag_cols = spool.tile([128, KTL], dt.float32, name="diag_cols")
            nc.sync.dma_start(diag_cols[:], d_out[:].rearrange("k p -> p k"))

            # ---- main matmul: out^T = V_half^T @ X^T, f32r on TensorE ----
            stage = stage_pool.tile([128, KTL * BPC], dt.float32, name="stage")
            for _rep in range(repeat):
              k0 = 0
              for kg, g in enumerate(KGROUPS):
                pss = [[pp.tile([128, 512], dt.float32, tag="ps",
                                name=f"ps{_rep}_{kg}_{q}_{b2}")
                        for b2 in range(2)] for q in range(g)]
                for j in range(FT):
                    vt = vpool.tile([128, 3 * 128], dt.float32r, tag="vt",
                                    name=f"vt{_rep}_{kg}_{j}")
                    nc.sync.dma_start(
                        vt[:, :g * 128],
                        v[j * 128:(j + 1) * 128, k0 * 128:(k0 + g) * 128])
                    for q in range(g):
                        for b2 in range(2):
                            nc.tensor.matmul(
                                pss[q][b2][:],
                                vt[:, q * 128:(q + 1) * 128],
                                x_all[:, j * BPC + b2 * 512:j * BPC + (b2 + 1) * 512],
                                start=(j == 0), stop=(j == FT - 1))
                for q in range(g):
                    kt = k0 + q
                    for b2 in range(2):
                        nc.vector.tensor_copy(
                            stage[:, kt * BPC + b2 * 512:kt * BPC + (b2 + 1) * 512],
                            pss[q][b2][:])
                    ot = opool.tile([128, BPC], dt.float32, tag="ot",
                                    name=f"ot{_rep}_{kt}")
                    nc.scalar.activation(ot[:], stage[:, kt * BPC:(kt + 1) * BPC],
                                         mybir.ActivationFunctionType.Tanh,
                                         bias=diag_cols[:, kt:kt + 1])
                    nc.sync.dma_start(out[kt * 128:(kt + 1) * 128, :], ot[:])
                k0 += g

    nc.compile()
    return nc


def _get_nc():
    if "nc" not in _CACHE:
        _CACHE["nc"] = _build_nc()
    return _CACHE["nc"]


def make_in_maps(e1, e2, W, V, b):
    in_maps = []
    for c in range(N_CORES):
        g, h = c // 2, c % 2
        sc = h * 4 + g            # permuted diag-slice index (see module doc)
        rows = slice(g * BPC, (g + 1) * BPC)
        krows = slice(sc * KPC, (sc + 1) * KPC)
        in_maps.append({
            "e1t": np.ascontiguousarray(e1[rows].T),
            "e2t": np.ascontiguousarray(e2[rows].T),
            "v": np.ascontiguousarray(V[:, h * KHC:(h + 1) * KHC]),
            "wt": np.ascontiguousarray(W[krows].T),
            "bvec": b[krows].reshape(1, KPC),
        })
    return in_maps


def kernel(e1, e2, W, V, b):
    from concourse.bass_utils import run_bass_kernel_spmd

    e1 = np.asarray(e1, dtype=np.float32)
    e2 = np.asarray(e2, dtype=np.float32)
    W = np.asarray(W, dtype=np.float32)
    V = np.asarray(V, dtype=np.float32)
    b = np.asarray(b, dtype=np.float32)

    nc = _get_nc()
    res = run_bass_kernel_spmd(nc, make_in_maps(e1, e2, W, V, b),
                               list(range(N_CORES)))
    out = np.empty((B, K_OUT), dtype=np.float32)
    for c in range(N_CORES):
        g, h = c // 2, c % 2
        out[g * BPC:(g + 1) * BPC, h * KHC:(h + 1) * KHC] = res.results[c]["out"].T
    return out
